# revision 1
# baseline (speedup 1.0000x reference)
"""Trainium2 Bass kernel for a pairwise-distance cluster margin loss.

Math (matches the jax reference):
    sq_i   = ||x_i||^2
    dist2  = sq_i + sq_j - 2 * x_i . x_j          (4096 x 4096)
    dist   = sqrt(max(dist2, eps))
    mask   = targets_i == targets_j
    far_i  = max_{j in class(i)} dist_ij
    near_i = second smallest dist_ij over class(i)  (smallest is self)
    loss   = mean(relu(far - near))

Strategy: row-shard the 4096 rows over 8 NeuronCores (512 rows each).
Each core streams the full x^T through its PE to produce, per
[128 x 512] PSUM tile,
    psA = dist2 + C*mask     (fp8e4m3 DoubleRow chain for the x part +
                              one bf16 aug matmul carrying sq hi/lo and
                              C*onehot class rows)
    psB = 2C*mask - 2^31*diag  (1-2 cheap bf16 matmuls)
On-chip reductions then give
    rowmax(psA)        = C + far2
    rowmax(psB - psA)  = C - near2   (diag pushed to -2^31, excluded)
and the host only applies max-over-slabs / sqrt / relu / mean to the
small reduced stats. fp8 quantization of x adds ~2e-4 relative error to
the loss (validated against an fp64 host model).

Each core's rhs slab order is rotated so its diagonal block is always
program-slab 0 - the diag fixup matmul is only emitted there (SPMD-safe,
no per-slab zero matmuls).
"""

import numpy as np
import ml_dtypes

BF = ml_dtypes.bfloat16
F8 = ml_dtypes.float8_e4m3

N = 4096  # rows (points)
D = 2048  # feature dim
P = 128  # partitions
NCORES = 8
MB = N // NCORES  # 512 rows per core
KX = D // P  # 16 x-chunks of 128
NT = N // 512  # 8 column tiles of 512
MT = MB // P  # 4 row tiles of 128 per core
NCLS = 64

C = float(2.0**17)  # mask offset; > max dist2 (~8.2k), keeps fp32 resolution
DIAG = -float(2.0**31)  # diagonal push-out in psB

_compiled = None


def _build_nc():
    import concourse.mybir as mybir
    import concourse.tile as tile
    from concourse import bacc
    from concourse.bass import ts

    nc = bacc.Bacc("TRN2", target_bir_lowering=False)
    f32 = mybir.dt.float32
    bf16 = mybir.dt.bfloat16
    fp8 = mybir.dt.float8e4
    DR = mybir.MatmulPerfMode.DoubleRow

    rhs8_d = nc.dram_tensor("rhs8", [NT, P, KX, 512], fp8, kind="ExternalInput")
    rhsa_d = nc.dram_tensor("rhsa", [NT, P, 512], bf16, kind="ExternalInput")
    lhs8_d = nc.dram_tensor("lhs8", [P, KX, MB], fp8, kind="ExternalInput")
    lhsaa_d = nc.dram_tensor("lhsaa", [P, MB], bf16, kind="ExternalInput")
    lhsb_d = nc.dram_tensor("lhsb", [P, MB], bf16, kind="ExternalInput")
    eye_d = nc.dram_tensor("eye", [P, P], bf16, kind="ExternalInput")
    dmat_d = nc.dram_tensor("dmat", [P, MT, 512], bf16, kind="ExternalInput")
    res_d = nc.dram_tensor("res", [MT, 2, P, NT], f32, kind="ExternalOutput")

    X = mybir.AxisListType.X

    with tile.TileContext(nc) as tc:
        with (
            tc.tile_pool(name="singles", bufs=1) as singles,
            tc.tile_pool(name="rhsp", bufs=3) as rhsp,
            tc.tile_pool(name="rhap", bufs=2) as rhap,
            tc.tile_pool(name="psa", bufs=5, space="PSUM") as psa,
            tc.tile_pool(name="psb", bufs=3, space="PSUM") as psb,
            tc.tile_pool(name="sbb", bufs=3) as sbb,
            tc.tile_pool(name="gsc", bufs=3) as gsc,
        ):
            lhs8 = singles.tile([P, KX, MB], fp8)
            rhs0 = rhsp.tile([P, KX, 512], fp8, name="rhs0")
            lhsb = singles.tile([P, MB], bf16)
            # smallest deps first: psB-mask matmuls can start on these alone
            nc.sync.dma_start(out=lhsb, in_=lhsb_d[:, :])
            rha0 = rhap.tile([P, 512], bf16, name="rha0")
            nc.sync.dma_start(out=rha0, in_=rhsa_d[0])
            eye = singles.tile([P, P], bf16)
            nc.sync.dma_start(out=eye, in_=eye_d[:, :])
            dmat = singles.tile([P, MT, 512], bf16)
            nc.sync.dma_start(out=dmat, in_=dmat_d[:, :, :])
            # interleave so the first psA chain's deps land earliest
            nc.sync.dma_start(out=lhs8[:, 0:1, :], in_=lhs8_d[:, 0:1, :])
            nc.sync.dma_start(out=rhs0[:, 0:1, :], in_=rhs8_d[0, :, 0:1, :])
            nc.sync.dma_start(out=lhs8[:, 1:3, :], in_=lhs8_d[:, 1:3, :])
            nc.sync.dma_start(out=rhs0[:, 1:3, :], in_=rhs8_d[0, :, 1:3, :])
            nc.sync.dma_start(out=lhs8[:, 3:5, :], in_=lhs8_d[:, 3:5, :])
            nc.sync.dma_start(out=rhs0[:, 3:5, :], in_=rhs8_d[0, :, 3:5, :])
            nc.sync.dma_start(out=lhs8[:, 5:7, :], in_=lhs8_d[:, 5:7, :])
            nc.sync.dma_start(out=rhs0[:, 5:7, :], in_=rhs8_d[0, :, 5:7, :])
            nc.sync.dma_start(out=lhs8[:, 7:10, :], in_=lhs8_d[:, 7:10, :])
            nc.sync.dma_start(out=rhs0[:, 7:10, :], in_=rhs8_d[0, :, 7:10, :])
            nc.sync.dma_start(out=lhs8[:, 10:13, :], in_=lhs8_d[:, 10:13, :])
            nc.sync.dma_start(out=rhs0[:, 10:13, :], in_=rhs8_d[0, :, 10:13, :])
            nc.sync.dma_start(out=lhs8[:, 13:KX, :], in_=lhs8_d[:, 13:KX, :])
            nc.sync.dma_start(out=rhs0[:, 13:KX, :], in_=rhs8_d[0, :, 13:KX, :])
            lhsaa = singles.tile([P, MB], bf16)
            nc.sync.dma_start(out=lhsaa, in_=lhsaa_d[:, :])

            fstats = [
                singles.tile([P, NT], f32, tag=f"fs{m}", name=f"fs{m}")
                for m in range(MT)
            ]
            gstats = [
                singles.tile([P, NT], f32, tag=f"gs{m}", name=f"gs{m}")
                for m in range(MT)
            ]

            for s in range(NT):
                if s == 0:
                    rhs = rhs0
                    rha = rha0
                else:
                    rhs = rhsp.tile([P, KX, 512], fp8, tag="rhs0", name="rhsl")
                    nc.sync.dma_start(out=rhs[:, 0:8, :], in_=rhs8_d[s, :, 0:8, :])
                    nc.sync.dma_start(out=rhs[:, 8:KX, :], in_=rhs8_d[s, :, 8:KX, :])
                    rha = rhap.tile([P, 512], bf16, tag="rha0", name="rhal")
                    nc.sync.dma_start(out=rha, in_=rhsa_d[s])

                for mt in range(MT):
                    # psB first: its deps are tiny, keeps PE busy during the
                    # initial x-chunk DMA
                    b = psb.tile([P, 512], f32)
                    nc.tensor.matmul(
                        b, lhsb[:, ts(mt, P)], rha, start=True, stop=(s != 0)
                    )
                    if s == 0:
                        nc.tensor.matmul(
                            b, eye, dmat[:, mt, :], start=False, stop=True
                        )
                    a = psa.tile([P, 512], f32)
                    if s == 0 and mt == 0:
                        # solo chunk 0/15 (non-DR) only for the very first
                        # tile: the first matmul then needs just one 128KB
                        # DMA landed, at the price of one extra instruction
                        nc.tensor.matmul(
                            a, lhs8[:, 0, ts(mt, P)], rhs[:, 0, :],
                            start=True, stop=False,
                        )
                        for c in range(1, KX - 1, 2):
                            nc.tensor.matmul(
                                a,
                                lhs8[:, c : c + 2, ts(mt, P)],
                                rhs[:, c : c + 2, :],
                                start=False,
                                stop=False,
                                perf_mode=DR,
                            )
                        nc.tensor.matmul(
                            a, lhs8[:, KX - 1, ts(mt, P)], rhs[:, KX - 1, :],
                            start=False, stop=False,
                        )
                    else:
                        for c in range(0, KX, 2):
                            nc.tensor.matmul(
                                a,
                                lhs8[:, c : c + 2, ts(mt, P)],
                                rhs[:, c : c + 2, :],
                                start=(c == 0),
                                stop=False,
                                perf_mode=DR,
                            )
                    nc.tensor.matmul(
                        a, lhsaa[:, ts(mt, P)], rha, start=False, stop=True
                    )
                    bb = sbb.tile([P, 512], f32)
                    nc.scalar.copy(bb, b)
                    nc.vector.reduce_max(fstats[mt][:, s : s + 1], a, axis=X)
                    # tensor_tensor_reduce would fuse these two, but that
                    # raw-ISA op dies on this compile path (NRT exec error)
                    g = gsc.tile([P, 512], f32)
                    nc.vector.tensor_sub(g, bb, a)
                    nc.vector.reduce_max(gstats[mt][:, s : s + 1], g, axis=X)

            for mt in range(MT):
                nc.sync.dma_start(out=res_d[mt, 0], in_=fstats[mt])
                nc.sync.dma_start(out=res_d[mt, 1], in_=gstats[mt])

    nc.compile()
    return nc


def _prep_inputs(x, t):
    """Host-side encode of the operands (x parts fp8, aug rows bf16)."""
    x = np.asarray(x, np.float32)
    t = np.asarray(t).astype(np.int64)
    sq = np.sum(x.astype(np.float64) ** 2, axis=1)
    sqhi = sq.astype(BF)
    sqlo = (sq - sqhi.astype(np.float64)).astype(BF)

    ohT = np.zeros((NCLS, N), BF)
    ohT[t, np.arange(N)] = BF(1.0)

    # fp8 x parts
    R8 = np.ascontiguousarray((-2.0 * x).astype(F8).T).reshape(KX, P, N)
    rhs8_np = np.ascontiguousarray(R8.reshape(KX, P, NT, 512).transpose(2, 1, 0, 3))
    L8 = np.ascontiguousarray(x.astype(F8).T).reshape(KX, P, N)

    # bf16 aug chunk: [sq_hi ; sq_lo ; 1 ; 1 ; C*onehot ; 0...]
    RA = np.zeros((P, N), BF)
    RA[0] = sqhi
    RA[1] = sqlo
    RA[2] = BF(1.0)
    RA[3] = BF(1.0)
    RA[4 : 4 + NCLS] = (C * ohT.astype(np.float32)).astype(BF)
    rhsa_np = np.ascontiguousarray(RA.reshape(P, NT, 512).transpose(1, 0, 2))

    LAA = np.zeros((P, N), BF)  # psA aug lhs: [1 ; 1 ; sq_hi ; sq_lo ; onehot]
    LAA[0] = BF(1.0)
    LAA[1] = BF(1.0)
    LAA[2] = sqhi
    LAA[3] = sqlo
    LAA[4 : 4 + NCLS] = ohT

    LB = np.zeros((P, N), BF)  # psB aug lhs: [0;0;0;0; 2*onehot]
    LB[4 : 4 + NCLS] = (2.0 * ohT.astype(np.float32)).astype(BF)

    eye_np = np.zeros((P, P), BF)
    eye_np[np.arange(P), np.arange(P)] = BF(1.0)

    dmat = np.zeros((P, MT, 512), BF)
    for mt in range(MT):
        dmat[np.arange(P), mt, mt * P + np.arange(P)] = BF(DIAG)

    in_maps = []
    for c0 in range(NCORES):
        sl = slice(c0 * MB, (c0 + 1) * MB)
        l8 = np.ascontiguousarray(L8[:, :, sl].transpose(1, 0, 2))  # [P, KX, MB]
        laa = np.ascontiguousarray(LAA[:, sl])
        lb = np.ascontiguousarray(LB[:, sl])
        # rotate slabs: program slab s holds global tile (c0 + s) % NT, so
        # the diagonal block is always at program slab 0
        r8 = np.ascontiguousarray(np.roll(rhs8_np, -c0, axis=0))
        ra = np.ascontiguousarray(np.roll(rhsa_np, -c0, axis=0))
        in_maps.append(
            {
                "rhs8": r8,
                "rhsa": ra,
                "lhs8": l8,
                "lhsaa": laa,
                "lhsb": lb,
                "eye": eye_np,
                "dmat": dmat,
            }
        )
    return in_maps


def _assemble(results):
    far2 = np.empty(N, np.float64)
    near2 = np.empty(N, np.float64)
    for c0 in range(NCORES):
        r = np.asarray(results[c0]["res"], np.float64)  # [MT, 2, P, NT]
        fmax = r[:, 0].max(axis=2)  # [MT, P]
        gmax = r[:, 1].max(axis=2)
        for mt in range(MT):
            idx = c0 * MB + mt * P + np.arange(P)
            far2[idx] = fmax[mt] - C
            near2[idx] = C - gmax[mt]
    far = np.sqrt(np.maximum(far2, 0.0))
    near = np.sqrt(np.maximum(near2, 0.0))
    loss = np.float32(np.mean(np.maximum(far - near, 0.0)))
    return np.asarray(loss, np.float32)


def run_kernel(inputs, targets, trace=False):
    """Returns (loss, BassKernelResults)."""
    from concourse.bass_utils import run_bass_kernel_spmd

    global _compiled
    if _compiled is None:
        _compiled = _build_nc()
    nc = _compiled
    in_maps = _prep_inputs(inputs, targets)
    br = run_bass_kernel_spmd(
        nc, in_maps, core_ids=list(range(NCORES)), trace=trace
    )
    return _assemble(br.results), br


def kernel(inputs, targets):
    loss, _ = run_kernel(inputs, targets)
    return loss



# revision 4
# speedup vs baseline: 3.0839x; 3.0839x over previous
"""Trainium2 Bass kernel for a pairwise-distance cluster margin loss.

Math (matches the jax reference):
    far_i  = max_{j: t_j=t_i} dist_ij
    near_i = second smallest dist_ij over class(i)  (smallest is self)
    loss   = mean(relu(far - near))

Key insight: the loss only involves SAME-CLASS distances.  With rows
sorted by class, each 128-row tile's class-mates lie within a narrow
band of the sorted order (max class size ~82), so each tile only needs
W ~ 288 columns instead of 4096 -> ~14x less GEMM work than the full
distance matrix.

Per core (512 sorted rows): the column "universe" is the sorted slice
order[512c-SPL : 512c-SPL+NCOL] (padded with zeros at the array ends).
Row-tile mt multiplies against universe cols [128mt, 128mt+W).  A single
fp8 tensor xt8 = fp8(sqrt2*x[universe])^T serves as BOTH matmul operands
(lhsT slice = own rows, rhs slice = window), so the PE computes
    psA = 2 x_i.x_j - sq_j - C*mask      (fp8 DR chain + one bf16 aug)
    psB = 2C*mask + DIAG*diag            (2 cheap bf16 matmuls)
and the stats flip max<->min versus the usual formulation:
    rowmin(psA)       = -(C + far2 - sq_i)   -> far2 = sq_i - C - fstat
    rowmax(psA + psB) = C - near2 + sq_i     -> near2 = sq_i + C - gstat
Host applies sqrt / relu / mean on the 4096 reduced stats.
"""

import numpy as np
import ml_dtypes

BF = ml_dtypes.bfloat16
F8 = ml_dtypes.float8_e4m3

N = 4096  # rows (points)
D = 2048  # feature dim
P = 128  # partitions
NCORES = 8
MB = N // NCORES  # 512 rows per core
KX = D // P  # 16 x-chunks of 128
MT = MB // P  # 4 row tiles of 128 per core
NCLS = 64

C = float(2.0**17)  # mask offset; > max |2xixj - sqj| (~15k)
DIAG = -float(2.0**31)  # diagonal push-out in psB

_compiled = None  # (key, nc)


def _spill(ts):
    """Max class-band spill (left, right) over all 128-row windows of the
    class-sorted target vector ts."""
    spl = spr = 0
    nw = N // P
    for w in range(nw):
        lo_cls = ts[w * P]
        hi_cls = ts[w * P + P - 1]
        lo = int(np.searchsorted(ts, lo_cls, "left"))
        hi = int(np.searchsorted(ts, hi_cls, "right"))
        spl = max(spl, w * P - lo)
        spr = max(spr, hi - (w * P + P))
    return spl, spr


def _build_nc(SPL, W, NCOL):
    import concourse.mybir as mybir
    import concourse.tile as tile
    from concourse import bacc

    nc = bacc.Bacc("TRN2", target_bir_lowering=False)
    f32 = mybir.dt.float32
    bf16 = mybir.dt.bfloat16
    fp8 = mybir.dt.float8e4
    DR = mybir.MatmulPerfMode.DoubleRow
    X = mybir.AxisListType.X
    MIN = mybir.AluOpType.min

    xt_d = nc.dram_tensor("xt", [P, KX, NCOL], fp8, kind="ExternalInput")
    aug_d = nc.dram_tensor("aug", [P, NCOL], bf16, kind="ExternalInput")
    lhsa_d = nc.dram_tensor("lhsa", [P, MB], bf16, kind="ExternalInput")
    lhsb_d = nc.dram_tensor("lhsb", [P, MB], bf16, kind="ExternalInput")
    eye_d = nc.dram_tensor("eye", [P, P], bf16, kind="ExternalInput")
    dmat_d = nc.dram_tensor("dmat", [P, W], bf16, kind="ExternalInput")
    res_d = nc.dram_tensor("res", [2, P, MT], f32, kind="ExternalOutput")

    with tile.TileContext(nc) as tc:
        with (
            tc.tile_pool(name="singles", bufs=1) as singles,
            tc.tile_pool(name="psa", bufs=1, space="PSUM") as psa,
            tc.tile_pool(name="psb", bufs=1, space="PSUM") as psb,
            tc.tile_pool(name="sbb", bufs=2) as sbb,
            tc.tile_pool(name="gsc", bufs=2) as gsc,
        ):
            # tiny psB deps first so the PE has work during the xt DMA
            lhsb = singles.tile([P, MB], bf16)
            nc.sync.dma_start(out=lhsb, in_=lhsb_d[:, :])
            eye = singles.tile([P, P], bf16)
            nc.sync.dma_start(out=eye, in_=eye_d[:, :])
            dmat = singles.tile([P, W], bf16)
            nc.sync.dma_start(out=dmat, in_=dmat_d[:, :])
            aug = singles.tile([P, NCOL], bf16)
            nc.sync.dma_start(out=aug, in_=aug_d[:, :])
            lhsa = singles.tile([P, MB], bf16)
            nc.sync.dma_start(out=lhsa, in_=lhsa_d[:, :])
            xt = singles.tile([P, KX, NCOL], fp8)
            for c in range(0, KX, 2):
                nc.sync.dma_start(out=xt[:, c : c + 2, :], in_=xt_d[:, c : c + 2, :])

            fst = singles.tile([P, MT], f32, name="fst")
            gst = singles.tile([P, MT], f32, name="gst")

            bt = []
            for mt in range(MT):
                b = psb.tile([P, 512], f32, name=f"b{mt}")
                nc.tensor.matmul(
                    b[:, 0:W],
                    lhsb[:, mt * P : (mt + 1) * P],
                    aug[:, mt * P : mt * P + W],
                    start=True,
                    stop=False,
                )
                nc.tensor.matmul(b[:, 0:W], eye, dmat, start=False, stop=True)
                bt.append(b)

            for mt in range(MT):
                m0 = SPL + mt * P
                c0 = mt * P
                a = psa.tile([P, 512], f32, name=f"a{mt}")
                for c in range(0, KX, 2):
                    nc.tensor.matmul(
                        a[:, 0:W],
                        xt[:, c : c + 2, m0 : m0 + P],
                        xt[:, c : c + 2, c0 : c0 + W],
                        start=(c == 0),
                        stop=False,
                        perf_mode=DR,
                    )
                nc.tensor.matmul(
                    a[:, 0:W],
                    lhsa[:, mt * P : (mt + 1) * P],
                    aug[:, c0 : c0 + W],
                    start=False,
                    stop=True,
                )
                nc.vector.tensor_reduce(
                    fst[:, mt : mt + 1], a[:, 0:W], axis=X, op=MIN
                )
                bb = sbb.tile([P, W], f32)
                nc.scalar.copy(bb, bt[mt][:, 0:W])
                g = gsc.tile([P, W], f32)
                nc.vector.tensor_add(g, bb, a[:, 0:W])
                nc.vector.reduce_max(gst[:, mt : mt + 1], g, axis=X)

            nc.sync.dma_start(out=res_d[0], in_=fst)
            nc.sync.dma_start(out=res_d[1], in_=gst)

    nc.compile()
    return nc


def _prep(x, t):
    x = np.asarray(x, np.float32)
    t = np.asarray(t).astype(np.int64)
    order = np.argsort(t, kind="stable")
    ts = t[order]
    spl, spr = _spill(ts)
    W = ((P + spl + spr) + 31) // 32 * 32
    NCOL = MB + (W - P)
    SPL = spl

    q8 = (np.float32(np.sqrt(2.0)) * x).astype(F8)  # [N, D]
    sq = np.sum(x.astype(np.float64) ** 2, axis=1)
    sqhi = sq.astype(BF)
    sqlo = (sq - sqhi.astype(np.float64)).astype(BF)

    eye_np = np.zeros((P, P), BF)
    eye_np[np.arange(P), np.arange(P)] = BF(1.0)
    dmat = np.zeros((P, W), BF)
    dmat[np.arange(P), SPL + np.arange(P)] = BF(DIAG)

    in_maps = []
    meta = []
    for c0 in range(NCORES):
        u0 = c0 * MB - SPL
        uidx = np.arange(u0, u0 + NCOL)
        valid = (uidx >= 0) & (uidx < N)
        gu = order[np.clip(uidx, 0, N - 1)]
        tu = ts[np.clip(uidx, 0, N - 1)]

        xt_cols = q8[gu].T.copy()  # [D, NCOL]
        xt_cols[:, ~valid] = F8(0.0)
        xt_np = np.ascontiguousarray(
            xt_cols.reshape(KX, P, NCOL).transpose(1, 0, 2)
        )

        aug_np = np.zeros((P, NCOL), BF)
        aug_np[0] = np.where(valid, -sqhi[gu], BF(0.0))
        aug_np[1] = np.where(valid, -sqlo[gu], BF(0.0))
        oh = np.zeros((NCLS, NCOL), np.float32)
        oh[tu, np.arange(NCOL)] = 1.0
        oh[:, ~valid] = 0.0
        aug_np[2 : 2 + NCLS] = (-C * oh).astype(BF)

        rows = order[c0 * MB : (c0 + 1) * MB]
        ohr = np.zeros((NCLS, MB), np.float32)
        ohr[t[rows], np.arange(MB)] = 1.0
        lhsa_np = np.zeros((P, MB), BF)
        lhsa_np[0] = BF(1.0)
        lhsa_np[1] = BF(1.0)
        lhsa_np[2 : 2 + NCLS] = ohr.astype(BF)
        lhsb_np = np.zeros((P, MB), BF)
        lhsb_np[2 : 2 + NCLS] = (-2.0 * ohr).astype(BF)

        in_maps.append(
            {
                "xt": xt_np,
                "aug": aug_np,
                "lhsa": lhsa_np,
                "lhsb": lhsb_np,
                "eye": eye_np,
                "dmat": dmat,
            }
        )
        meta.append(rows)
    return in_maps, meta, sq, (SPL, W, NCOL)


def _assemble(results, meta, sq):
    far2 = np.empty(N, np.float64)
    near2 = np.empty(N, np.float64)
    for c0 in range(NCORES):
        r = np.asarray(results[c0]["res"], np.float64)  # [2, P, MT]
        rows = meta[c0]
        for mt in range(MT):
            g = rows[mt * P : (mt + 1) * P]
            far2[g] = sq[g] - C - r[0, :, mt]
            near2[g] = sq[g] + C - r[1, :, mt]
    far = np.sqrt(np.maximum(far2, 0.0))
    near = np.sqrt(np.maximum(near2, 0.0))
    loss = np.float32(np.mean(np.maximum(far - near, 0.0)))
    return np.asarray(loss, np.float32)


def run_kernel(inputs, targets, trace=False):
    """Returns (loss, BassKernelResults)."""
    from concourse.bass_utils import run_bass_kernel_spmd

    global _compiled
    in_maps, meta, sq, key = _prep(inputs, targets)
    if _compiled is None or _compiled[0] != key:
        _compiled = (key, _build_nc(*key))
    nc = _compiled[1]
    br = run_bass_kernel_spmd(
        nc, in_maps, core_ids=list(range(NCORES)), trace=trace
    )
    return _assemble(br.results, meta, sq), br


def kernel(inputs, targets):
    loss, _ = run_kernel(inputs, targets)
    return loss


# revision 10
# speedup vs baseline: 4.3122x; 1.3983x over previous
"""Trainium2 Bass kernel for a pairwise-distance cluster margin loss.

Math (matches the jax reference):
    far_i  = max_{j: t_j=t_i} dist_ij
    near_i = second smallest dist_ij over class(i)  (smallest is self)
    loss   = mean(relu(far - near))

Key insight: the loss only involves SAME-CLASS distances.  With rows
sorted by class, each 128-row tile's class-mates lie within a narrow
band of the sorted order (max class size ~82), so each tile only needs
W ~ 264 columns instead of 4096 -> ~14x less GEMM work than the full
distance matrix.

Per core (512 sorted rows): the column "universe" is the sorted slice
order[512c-SPL : 512c-SPL+NCOL] (padded with zeros at the array ends).
Row-tile mt multiplies against universe cols [128mt, 128mt+W).  A single
fp8 tensor xt8 = fp8(sqrt2*x[universe])^T serves as BOTH matmul operands
(lhsT slice = own rows, rhs slice = window), so the PE computes
    psA = 2 x_i.x_j - sq_j - C*mask      (fp8 DR chain + one bf16 aug)
and the stats flip max<->min versus the usual formulation:
    rowmin(psA)                   -> far2  = sq_i - C - fstat
    rowmax(psA + 2C*mask + Ddiag) -> near2 = sq_i + C - gstat
The mask/diag term is a host-precomputed bf16 SBUF tensor added on the
DVE (no second matmul chain, no scalar-engine copy).  Host applies
sqrt / relu / mean on the 4096 reduced stats.
"""

import numpy as np
import ml_dtypes

BF = ml_dtypes.bfloat16
F8 = ml_dtypes.float8_e4m3

N = 4096  # rows (points)
D = 2048  # feature dim
P = 128  # partitions
NCORES = 8
MB = N // NCORES  # 512 rows per core
KX = D // P  # 16 x-chunks of 128
MT = MB // P  # 4 row tiles of 128 per core
NCLS = 64

C = float(2.0**17)  # mask offset; > max |2xixj - sqj| (~15k)
DIAG = -float(2.0**31)  # diagonal push-out

_compiled = None  # (key, nc)


def _spill(ts):
    """Max class-band spill (left, right) over all 128-row windows of the
    class-sorted target vector ts."""
    spl = spr = 0
    nw = N // P
    for w in range(nw):
        lo_cls = ts[w * P]
        hi_cls = ts[w * P + P - 1]
        lo = int(np.searchsorted(ts, lo_cls, "left"))
        hi = int(np.searchsorted(ts, hi_cls, "right"))
        spl = max(spl, w * P - lo)
        spr = max(spr, hi - (w * P + P))
    return spl, spr


def _build_nc(SPL, W, NCOL):
    import concourse.mybir as mybir
    import concourse.tile as tile
    from concourse import bacc

    nc = bacc.Bacc("TRN2", target_bir_lowering=False)
    f32 = mybir.dt.float32
    bf16 = mybir.dt.bfloat16
    fp8 = mybir.dt.float8e4
    DR = mybir.MatmulPerfMode.DoubleRow
    X = mybir.AxisListType.X
    MIN = mybir.AluOpType.min

    # packed bf16 tensor: [lhsa (MB) | aug (NCOL) | psbp (MT*W)]
    LHSA = 0
    AUG = MB
    PSBP = MB + NCOL
    PK = MB + NCOL + MT * W

    xt_d = nc.dram_tensor("xt", [P, KX, NCOL], fp8, kind="ExternalInput")
    pk_d = nc.dram_tensor("pk", [P, PK], bf16, kind="ExternalInput")
    res_d = nc.dram_tensor("res", [P, 2 * MT], f32, kind="ExternalOutput")

    with tile.TileContext(nc) as tc:
        with (
            tc.tile_pool(name="singles", bufs=1) as singles,
            tc.tile_pool(name="psa", bufs=1, space="PSUM") as psa,
            tc.tile_pool(name="gsc", bufs=2) as gsc,
        ):
            xt = singles.tile([P, KX, NCOL], fp8)
            pk = singles.tile([P, PK], bf16)
            # parallel DMA triggers across engine queues; first xt piece
            # lands earliest so the PE can start
            nc.sync.dma_start(out=xt[:, 0:2, :], in_=xt_d[:, 0:2, :])
            nc.scalar.dma_start(out=pk, in_=pk_d[:, :])
            nc.sync.dma_start(out=xt[:, 2:7, :], in_=xt_d[:, 2:7, :])
            nc.scalar.dma_start(out=xt[:, 7:12, :], in_=xt_d[:, 7:12, :])
            nc.sync.dma_start(out=xt[:, 12:KX, :], in_=xt_d[:, 12:KX, :])

            fg = singles.tile([P, 2 * MT], f32, name="fg")

            for mt in range(MT):
                m0 = SPL + mt * P
                c0 = mt * P
                a = psa.tile([P, 512], f32, name=f"a{mt}")
                for c in range(0, KX, 2):
                    nc.tensor.matmul(
                        a[:, 0:W],
                        xt[:, c : c + 2, m0 : m0 + P],
                        xt[:, c : c + 2, c0 : c0 + W],
                        start=(c == 0),
                        stop=False,
                        perf_mode=DR,
                    )
                nc.tensor.matmul(
                    a[:, 0:W],
                    pk[:, LHSA + mt * P : LHSA + (mt + 1) * P],
                    pk[:, AUG + c0 : AUG + c0 + W],
                    start=False,
                    stop=True,
                )
                g = gsc.tile([P, W], f32)
                nc.vector.tensor_add(
                    g, pk[:, PSBP + mt * W : PSBP + (mt + 1) * W], a[:, 0:W]
                )
                nc.vector.reduce_max(fg[:, MT + mt : MT + mt + 1], g, axis=X)
                nc.vector.tensor_reduce(
                    fg[:, mt : mt + 1], a[:, 0:W], axis=X, op=MIN
                )

            nc.sync.dma_start(out=res_d[:, :], in_=fg)

    nc.compile()
    return nc


def _prep(x, t):
    x = np.asarray(x, np.float32)
    t = np.asarray(t).astype(np.int64)
    order = np.argsort(t, kind="stable")
    ts = t[order]
    spl, spr = _spill(ts)
    W = ((P + spl + spr) + 7) // 8 * 8
    NCOL = (MB + (W - P) + 63) // 64 * 64  # 64-col aligned for LDWEIGHTS
    SPL = spl

    q8 = (np.float32(np.sqrt(2.0)) * x).astype(F8)  # [N, D]
    sq = np.sum(x.astype(np.float64) ** 2, axis=1)
    sqhi = sq.astype(BF)
    sqlo = (sq - sqhi.astype(np.float64)).astype(BF)

    LHSA = 0
    AUG = MB
    PSBP = MB + NCOL
    PK = MB + NCOL + MT * W

    in_maps = []
    meta = []
    for c0 in range(NCORES):
        u0 = c0 * MB - SPL
        uidx = np.arange(u0, u0 + NCOL)
        valid = (uidx >= 0) & (uidx < N)
        gu = order[np.clip(uidx, 0, N - 1)]
        tu = np.where(valid, ts[np.clip(uidx, 0, N - 1)], -1)

        xt_cols = q8[gu].T.copy()  # [D, NCOL]
        xt_cols[:, ~valid] = F8(0.0)
        xt_np = np.ascontiguousarray(
            xt_cols.reshape(KX, P, NCOL).transpose(1, 0, 2)
        )

        pk_np = np.zeros((P, PK), BF)
        # lhsa block: row0 = row1 = 1, rows 2+c = onehot(t_row)
        rows = order[c0 * MB : (c0 + 1) * MB]
        ohr = np.zeros((NCLS, MB), np.float32)
        ohr[t[rows], np.arange(MB)] = 1.0
        pk_np[0, LHSA : LHSA + MB] = BF(1.0)
        pk_np[1, LHSA : LHSA + MB] = BF(1.0)
        pk_np[2 : 2 + NCLS, LHSA : LHSA + MB] = ohr.astype(BF)
        # aug block: row0 = -sqhi_j, row1 = -sqlo_j, rows 2+c = -C*onehot
        pk_np[0, AUG : AUG + NCOL] = np.where(valid, -sqhi[gu], BF(0.0))
        pk_np[1, AUG : AUG + NCOL] = np.where(valid, -sqlo[gu], BF(0.0))
        oh = np.zeros((NCLS, NCOL), np.float32)
        oh[tu[valid], np.nonzero(valid)[0]] = 1.0
        pk_np[2 : 2 + NCLS, AUG : AUG + NCOL] = (-C * oh).astype(BF)
        # psbp blocks: per tile, 2C*mask with diag overwritten to DIAG
        for mt in range(MT):
            tr = tu[SPL + mt * P : SPL + mt * P + P]  # row classes
            tc_ = tu[mt * P : mt * P + W]  # window col classes
            m = (tr[:, None] == tc_[None, :]) & (tr[:, None] >= 0)
            blk = np.where(m, np.float32(2.0 * C), np.float32(0.0))
            blk[np.arange(P), SPL + np.arange(P)] = np.float32(DIAG)
            pk_np[:, PSBP + mt * W : PSBP + (mt + 1) * W] = blk.astype(BF)

        in_maps.append({"xt": xt_np, "pk": pk_np})
        meta.append(rows)
    return in_maps, meta, sq, (SPL, W, NCOL)


def _assemble(results, meta, sq):
    far2 = np.empty(N, np.float64)
    near2 = np.empty(N, np.float64)
    for c0 in range(NCORES):
        r = np.asarray(results[c0]["res"], np.float64)  # [P, 2*MT]
        rows = meta[c0]
        for mt in range(MT):
            g = rows[mt * P : (mt + 1) * P]
            far2[g] = sq[g] - C - r[:, mt]
            near2[g] = sq[g] + C - r[:, MT + mt]
    far = np.sqrt(np.maximum(far2, 0.0))
    near = np.sqrt(np.maximum(near2, 0.0))
    loss = np.float32(np.mean(np.maximum(far - near, 0.0)))
    return np.asarray(loss, np.float32)


def run_kernel(inputs, targets, trace=False):
    """Returns (loss, BassKernelResults)."""
    from concourse.bass_utils import run_bass_kernel_spmd

    global _compiled
    in_maps, meta, sq, key = _prep(inputs, targets)
    if _compiled is None or _compiled[0] != key:
        _compiled = (key, _build_nc(*key))
    nc = _compiled[1]
    br = run_bass_kernel_spmd(
        nc, in_maps, core_ids=list(range(NCORES)), trace=trace
    )
    return _assemble(br.results, meta, sq), br


def kernel(inputs, targets):
    loss, _ = run_kernel(inputs, targets)
    return loss
